# revision 11
# baseline (speedup 1.0000x reference)
"""CNN attention (nn_CNNAttention_77979426226593) Trainium2 Bass kernel.

Data-parallel over batch: B=16 images -> 8 NeuronCores, 2 images per core.
Each core holds the full (small) conv1x1 weights and computes its local
N x N attention (N = H*W = 4096) independently.

Per image (C=256, N=4096, CQK=32):
  q = wq @ x + bq            [32, N]   (4x-replicated across partitions)
  k = wk @ x + bk            [32, N]   (4x-replicated across partitions)
  vt = x^T @ (g*wv)^T + g*bv [N, 256]  (V transposed, gamma pre-folded)
  T[n, m] = k_n . q_m        (scores, transposed layout -> no transposes)
  E = exp(T)                 (no max-subtraction: logits are small by
                              construction, exp fits fp32/bf16 easily)
  U[c, m] = sum_n vt[n, c] * E[n, m]
  d[m]    = sum_n E[n, m]
  out[c, m] = U[c, m] / d[m] + x[c, m]

Matmuls run in bf16 with fp32 PSUM accumulation. gamma is folded into the
V weights host-side, so when gamma == 0 the attention term is exactly zero
and the output equals the fp32 residual copy of x bit-exactly.

v3 engine orchestration (from NTFF trace analysis of v1/v2):
- PE does only the real GEMMs: packed projections (q/k weights replicated
  4x inside one lhsT), 4-way row-tiled score matmuls, U matmuls, and eight
  f=512 ones-matmuls per image that finish the softmax denominator.
- The denominator partials never touch the PE during the m-tile loop:
  each exp'd chunk [128,512] is accumulated in-place into two SBUF
  accumulators (3 chunks/quad on VectorE, 1 on GpSimd) -- 32 adds/m-tile,
  the information-theoretic minimum.  No partition_all_reduce (it forced
  a GpSimd ucode library swap every m-tile), no per-m-tile reciprocal.
- Per m-tile: one VectorE fold (dacc_v+dacc_g -> bf16) and two ScalarE
  Copy evictions of the U accumulators (ScalarE has slack; the in-order
  DVE queue stays short so the exp -> score-matmul chain never blocks).
- Per image: 8 PE ones-matmuls reduce the folded dacc over partitions
  into one PSUM bank borrowed from the (closed) projection pools, a
  single reciprocal, then per-m-tile broadcast + 2 fused mult/add + DMA.
  All of it overlaps the next image's projections/attention.
- Projection evictions (bias add) run on ScalarE (Identity activation,
  per-partition bias); V evictions on GpSimd tensor_tensor.
"""

import numpy as np

B, C, H, W = 16, 256, 64, 64
N = H * W          # 4096
CQK = 32
NCORES = 8
BPC = B // NCORES  # batches per core

MT = 512           # m tile (attention output columns per PSUM tile)
NMT = N // MT      # 8
NCH = N // 128     # 32 n-chunks (contraction for U)
NQ = NCH // 4      # 8 quads per m-tile
NQTOT = NMT * NQ   # 64 quads per image


def _build_nc(repeat=1):
    import contextlib
    import concourse.bacc as bacc
    import concourse.mybir as mybir
    import concourse.tile as tile
    import concourse.bass as bass

    f32 = mybir.dt.float32
    bf16 = mybir.dt.bfloat16
    AF = mybir.ActivationFunctionType
    OP = mybir.AluOpType

    nc = bacc.Bacc("TRN2", target_bir_lowering=False, debug=False,
                   num_devices=NCORES)

    xb_d = nc.dram_tensor("xb", [BPC, C, N], bf16, kind="ExternalInput")
    xf_d = nc.dram_tensor("xf", [BPC, C, N], f32, kind="ExternalInput")
    wqT_d = nc.dram_tensor("wqT", [C, CQK], bf16, kind="ExternalInput")
    wkT_d = nc.dram_tensor("wkT", [C, CQK], bf16, kind="ExternalInput")
    wvT_d = nc.dram_tensor("wvT", [C, C], bf16, kind="ExternalInput")
    bq_d = nc.dram_tensor("bq", [CQK], f32, kind="ExternalInput")
    bk_d = nc.dram_tensor("bk", [CQK], f32, kind="ExternalInput")
    bv_d = nc.dram_tensor("bv", [C], bf16, kind="ExternalInput")
    ones_d = nc.dram_tensor("ones", [CQK], bf16, kind="ExternalInput")
    out_d = nc.dram_tensor("out", [BPC, C, N], f32, kind="ExternalOutput")

    def bcast_ap(handle, parts, free):
        # DRAM source AP replicated across `parts` partitions (step 0)
        return bass.AP(tensor=handle, offset=0, ap=[[0, parts], [1, free]])

    with tile.TileContext(nc) as tc:
        ctx = contextlib.ExitStack()
        with ctx:
            singles = ctx.enter_context(tc.tile_pool(name="singles", bufs=1))
            xpool = ctx.enter_context(tc.tile_pool(name="xpool", bufs=2))
            qkpool = ctx.enter_context(tc.tile_pool(name="qkpool", bufs=2))
            vtpool = ctx.enter_context(tc.tile_pool(name="vtpool", bufs=2))
            epool = ctx.enter_context(tc.tile_pool(name="epool", bufs=3))
            dvpool = ctx.enter_context(tc.tile_pool(name="dvpool", bufs=2))
            dgpool = ctx.enter_context(tc.tile_pool(name="dgpool", bufs=2))
            dapool = ctx.enter_context(tc.tile_pool(name="dapool", bufs=2))
            ucpool = ctx.enter_context(tc.tile_pool(name="ucpool", bufs=2))
            opool = ctx.enter_context(tc.tile_pool(name="opool", bufs=2))
            xrpool = ctx.enter_context(tc.tile_pool(name="xrpool", bufs=2))
            rpool = ctx.enter_context(tc.tile_pool(name="rpool", bufs=2))

            # --- constants / weights (once) ---
            # q/k weights packed 4x along lhsT free dim: one matmul pair
            # emits the 4x-replicated projection the row-tiled score
            # matmuls need, instead of four col-tiled pairs.
            wq4 = singles.tile([128, 2, 128], bf16, tag="wq4")
            wk4 = singles.tile([128, 2, 128], bf16, tag="wk4")
            for h in range(2):
                nc.gpsimd.dma_start(out=wq4[:, h, :], in_=bass.AP(
                    tensor=wqT_d, offset=h * 128 * CQK,
                    ap=[[CQK, 128], [0, 4], [1, CQK]]))
                nc.gpsimd.dma_start(out=wk4[:, h, :], in_=bass.AP(
                    tensor=wkT_d, offset=h * 128 * CQK,
                    ap=[[CQK, 128], [0, 4], [1, CQK]]))
            wvT = singles.tile([C // 2, 2, C], bf16, tag="wvT")
            nc.gpsimd.dma_start(out=wvT, in_=wvT_d.ap().rearrange(
                "(t p) o -> p t o", p=128))
            bq_sb = singles.tile([128, 1], f32, tag="bq")
            nc.gpsimd.dma_start(out=bq_sb, in_=bass.AP(
                tensor=bq_d, offset=0, ap=[[0, 4], [1, CQK]]))
            bk_sb = singles.tile([128, 1], f32, tag="bk")
            nc.gpsimd.dma_start(out=bk_sb, in_=bass.AP(
                tensor=bk_d, offset=0, ap=[[0, 4], [1, CQK]]))
            bv_b = singles.tile([1, C], bf16, tag="bvb")
            nc.gpsimd.dma_start(out=bv_b, in_=bcast_ap(bv_d, 1, C))
            ones_row = singles.tile([1, 128], bf16, tag="ones_r")
            nc.gpsimd.dma_start(out=ones_row, in_=bass.AP(
                tensor=ones_d, offset=0, ap=[[0, 1], [0, 4], [1, CQK]]))
            ones32 = singles.tile([128, 32], bf16, tag="ones32")
            nc.gpsimd.dma_start(out=ones32, in_=bass.AP(
                tensor=ones_d, offset=0, ap=[[0, 128], [1, 32]]))

            def body():
                # per-image deferred state: dall (folded denominator) and
                # uc (evicted U accumulators), consumed one image later
                img = {}

                def finalize(b, psum_pool):
                    """Per-image epilogue: reduce dall over partitions on
                    the PE (borrowed PSUM banks; out base partition must be
                    0/32/64, so 3 m-tiles per bank via a 32-wide ones
                    lhsT), one reciprocal per bank, then normalize +
                    residual-add + store each m-tile."""
                    s_ = img.pop(b)
                    dall, uc = s_["dall"], s_["uc"]
                    for t in range(3):
                        mts = [mt for mt in range(NMT) if mt // 3 == t]
                        dp = psum_pool.tile([128, MT], f32, tag="dpimg",
                                            name=f"dp_{b}_{t}")
                        for s, mt in enumerate(mts):
                            nc.tensor.matmul(dp[32 * s:32 * (s + 1), :],
                                             ones32, dall[:, mt, :],
                                             start=True, stop=True)
                        r_t = rpool.tile([96, MT], f32, tag="rimg",
                                         name=f"ri_{b}_{t}")
                        nc.vector.reciprocal(r_t[0:32 * len(mts), :],
                                             dp[0:32 * len(mts), :])
                        for s, mt in enumerate(mts):
                            ms = slice(mt * MT, (mt + 1) * MT)
                            xr = [xrpool.tile([128, MT], f32, tag=f"xr{h}",
                                              name=f"xr_{b}_{mt}_{h}")
                                  for h in range(2)]
                            for h in range(2):
                                nc.sync.dma_start(
                                    out=xr[h],
                                    in_=xf_d[b, 128 * h:128 * (h + 1), ms])
                            r128 = rpool.tile([128, MT], f32, tag="r128",
                                              name=f"r_{b}_{mt}")
                            nc.gpsimd.partition_broadcast(
                                r128, r_t[32 * s:32 * s + 1, :])
                            for h in range(2):
                                t1 = opool.tile([128, MT], f32, tag="t1")
                                nc.vector.tensor_tensor(out=t1,
                                                        in0=uc[:, h, mt, :],
                                                        in1=r128, op=OP.mult)
                                ot = opool.tile([128, MT], f32, tag="ot")
                                nc.vector.tensor_tensor(out=ot, in0=t1,
                                                        in1=xr[h], op=OP.add)
                                nc.sync.dma_start(
                                    out=out_d[b, 128 * h:128 * (h + 1), ms],
                                    in_=ot)

                for b in range(BPC):
                    # --- load x (bf16 compute copy) ---
                    xt = [xpool.tile([128, N], bf16, tag=f"x{h}",
                                     name=f"xt{h}_{b}") for h in range(2)]
                    for h in range(2):
                        nc.sync.dma_start(
                            out=xt[h], in_=xb_d[b, 128 * h:128 * (h + 1), :])

                    q_sb = qkpool.tile([128, N], bf16, tag="q")
                    k_sb = qkpool.tile([128, N], bf16, tag="k")
                    vt_sb = vtpool.tile([128, NCH, C], bf16, tag="vt")
                    img[b] = {
                        "dall": dapool.tile([128, NMT, MT], bf16, tag="dall",
                                            name=f"dall_{b}"),
                        "uc": ucpool.tile([128, 2, NMT, MT], bf16, tag="uc",
                                          name=f"uc_{b}"),
                    }

                    # --- projections (+ previous image's epilogue, which
                    # borrows a projection-pool PSUM bank for its d-matmuls
                    # and overlaps this image's compute) ---
                    with tc.tile_pool(name="ppsum", bufs=2, space="PSUM") as pp, \
                         tc.tile_pool(name="vpsum", bufs=2, space="PSUM") as vp_:
                        for nt in range(NMT):
                            ns = slice(nt * MT, (nt + 1) * MT)
                            qp = pp.tile([128, MT], f32, tag="qp")
                            for h in range(2):
                                nc.tensor.matmul(qp, wq4[:, h, :], xt[h][:, ns],
                                                 start=(h == 0), stop=(h == 1))
                            nc.scalar.activation(q_sb[:, ns], qp, AF.Identity,
                                                 bias=bq_sb)
                            kp = pp.tile([128, MT], f32, tag="kp")
                            for h in range(2):
                                nc.tensor.matmul(kp, wk4[:, h, :], xt[h][:, ns],
                                                 start=(h == 0), stop=(h == 1))
                            nc.scalar.activation(k_sb[:, ns], kp, AF.Identity,
                                                 bias=bk_sb)
                        for ni in range(NCH):
                            cs = slice(ni * 128, (ni + 1) * 128)
                            vp = vp_.tile([128, C], f32, tag="vp")
                            for h in range(2):
                                nc.tensor.matmul(vp, xt[h][:, cs], wvT[:, h, :],
                                                 start=(h == 0), stop=False)
                            nc.tensor.matmul(vp, ones_row, bv_b,
                                             start=False, stop=True)
                            nc.vector.tensor_copy(vt_sb[:, ni, :], vp)
                        if b > 0:
                            finalize(b - 1, pp)

                    # --- attention: 64 quads, software-pipelined across
                    # m-tiles (u-accumulators double-buffered in PSUM) ---
                    with tc.tile_pool(name="u0psum", bufs=2, space="PSUM") as up0, \
                         tc.tile_pool(name="u1psum", bufs=2, space="PSUM") as up1, \
                         tc.tile_pool(name="tpsum", bufs=1, space="PSUM") as tpp:
                        st = {}

                        def t_stage(qi):
                            mt, g = divmod(qi, NQ)
                            ms = slice(mt * MT, (mt + 1) * MT)
                            if g == 0:
                                st[mt] = {
                                    "u0": up0.tile([128, MT], f32, tag="u0",
                                                   name=f"u0_{b}_{mt}"),
                                    "u1": up1.tile([128, MT], f32, tag="u1",
                                                   name=f"u1_{b}_{mt}"),
                                }
                            tp = tpp.tile([128, 4, MT], f32, tag="tp",
                                          name=f"tp_{b}_{qi}")
                            for j in range(4):
                                ni = 4 * g + j
                                nc.tensor.matmul(
                                    tp[:, j, :],
                                    k_sb[32 * j:32 * (j + 1),
                                         ni * 128:(ni + 1) * 128],
                                    q_sb[32 * j:32 * (j + 1), ms],
                                    start=True, stop=True,
                                    tile_position=(32 * j, 0))
                            st[("tp", qi)] = tp

                        def e_stage(qi):
                            mt, g = divmod(qi, NQ)
                            tp = st.pop(("tp", qi))
                            e = epool.tile([128, 4, MT], bf16, tag="e",
                                           name=f"e_{b}_{qi}")
                            # two halves: the next quad's score matmuls only
                            # wait on the half that frees their PSUM bank
                            nc.scalar.activation(e[:, 0:2, :], tp[:, 0:2, :],
                                                 AF.Exp)
                            nc.scalar.activation(e[:, 2:4, :], tp[:, 2:4, :],
                                                 AF.Exp)
                            # softmax denominator: accumulate the four
                            # exp'd chunks straight into two in-place
                            # accumulators (3 on VectorE, 1 on GpSimd)
                            if g == 0:
                                dv = st[mt]["dv"] = dvpool.tile(
                                    [128, 2, MT], f32, tag="dv",
                                    name=f"dv_{b}_{mt}")
                                dg = st[mt]["dg"] = dgpool.tile(
                                    [128, 2, MT], f32, tag="dg",
                                    name=f"dg_{b}_{mt}")
                                nc.vector.tensor_copy(dv, e[:, 0:2, :])
                                nc.gpsimd.tensor_copy(dg, e[:, 2:4, :])
                            else:
                                dv, dg = st[mt]["dv"], st[mt]["dg"]
                                nc.vector.tensor_tensor(out=dv, in0=dv,
                                                        in1=e[:, 0:2, :],
                                                        op=OP.add)
                                nc.gpsimd.tensor_tensor(out=dg, in0=dg,
                                                        in1=e[:, 2:4, :],
                                                        op=OP.add)
                            st[("e", qi)] = e

                        def u_stage(qi):
                            mt, g = divmod(qi, NQ)
                            e = st.pop(("e", qi))
                            u0, u1 = st[mt]["u0"], st[mt]["u1"]
                            for j in range(4):
                                ni = 4 * g + j
                                stt = ni == 0
                                spp = ni == NCH - 1
                                ej = e[:, j, :]
                                nc.tensor.matmul(u0, vt_sb[:, ni, 0:128],
                                                 ej, start=stt, stop=spp)
                                nc.tensor.matmul(u1, vt_sb[:, ni, 128:256],
                                                 ej, start=stt, stop=spp)

                        def mt_end(mt):
                            s_ = st.pop(mt)
                            # fold the two accumulators into the per-image
                            # denominator buffer (bf16)
                            ft = dvpool.tile([128, 2, MT], f32, tag="ft",
                                             name=f"ft_{b}_{mt}")
                            nc.vector.tensor_tensor(out=ft, in0=s_["dv"],
                                                    in1=s_["dg"], op=OP.add)
                            nc.vector.tensor_tensor(
                                out=img[b]["dall"][:, mt, :], in0=ft[:, 0, :],
                                in1=ft[:, 1, :], op=OP.add)
                            # evict U accumulators (2x-mode DVE copies)
                            for h, u in ((0, s_["u0"]), (1, s_["u1"])):
                                nc.vector.tensor_copy(
                                    img[b]["uc"][:, h, mt, :], u)

                        for qi in range(NQTOT):
                            t_stage(qi)
                            if qi >= 1:
                                e_stage(qi - 1)
                            if qi >= 2:
                                u_stage(qi - 2)
                                if (qi - 2) % NQ == NQ - 1:
                                    mt_end((qi - 2) // NQ)
                        e_stage(NQTOT - 1)
                        u_stage(NQTOT - 2)
                        u_stage(NQTOT - 1)
                        mt_end(NMT - 1)

                # epilogue for the last image
                with tc.tile_pool(name="fpsum", bufs=1, space="PSUM") as fp:
                    finalize(BPC - 1, fp)

            if repeat == 1:
                body()
            else:
                with tc.For_i(0, repeat, 1):
                    body()

    nc.finalize()
    return nc


_NC_CACHE = {}


def _get_nc():
    if "nc" not in _NC_CACHE:
        _NC_CACHE["nc"] = _build_nc()
    return _NC_CACHE["nc"]


def make_in_maps(inputs, wq, bq, wk, bk, wv, bv, gamma):
    import ml_dtypes
    bf16 = ml_dtypes.bfloat16

    x = np.ascontiguousarray(np.asarray(inputs, np.float32).reshape(B, C, N))
    xb = x.astype(bf16)
    g = float(np.asarray(gamma, np.float32).reshape(-1)[0])
    wqT = np.ascontiguousarray(np.asarray(wq, np.float32).T).astype(bf16)
    wkT = np.ascontiguousarray(np.asarray(wk, np.float32).T).astype(bf16)
    # gamma folded into V: U = (g*wv) x E  =>  out = U/d + x
    wvT = np.ascontiguousarray(
        (np.asarray(wv, np.float32) * g).T).astype(bf16)
    bq = np.asarray(bq, np.float32)
    bk = np.asarray(bk, np.float32)
    bv = (np.asarray(bv, np.float32) * g).astype(bf16)

    in_maps = []
    for c in range(NCORES):
        sl = slice(c * BPC, (c + 1) * BPC)
        in_maps.append({
            "xb": xb[sl], "xf": x[sl],
            "wqT": wqT, "wkT": wkT, "wvT": wvT,
            "bq": bq, "bk": bk, "bv": bv,
            "ones": np.ones(CQK, bf16),
        })
    return in_maps


def kernel(inputs, wq, bq, wk, bk, wv, bv, gamma):
    from concourse.bass_utils import run_bass_kernel_spmd

    nc = _get_nc()
    in_maps = make_in_maps(inputs, wq, bq, wk, bk, wv, bv, gamma)
    res = run_bass_kernel_spmd(nc, in_maps, core_ids=list(range(NCORES)))
    out = np.concatenate([res.results[c]["out"] for c in range(NCORES)], axis=0)
    return out.reshape(B, C, H, W)
